# revision 1
# baseline (speedup 1.0000x reference)
"""Trainium2 Bass kernel for nn_GPT_61409442398424 (4-layer spiking GPT).

Sharding: DP-2 over batch (core groups {0-3},{4-7}) x TP-4 within group
(Wq/Wk/Wv by heads, Wfc/Wp by hidden dim, uni by HASH rows for logits).

v2 design notes:
- QKV computed TRANSPOSED ([qkv_dim, tokens]) in f32r at full PE rate; the
  rope half-swap comes from a permutation matmul, so no DVE transposes.
- The pre-attention rmsnorm of x cancels inside the per-head q/k rmsnorms
  (rmsnorm is scale-invariant per token); v's share of it and the k-head
  norm are folded into the softmax exp() as per-key scale/bias APs.
- LIF fixpoint: u = linear scan, then K=11 passes of
    e = (u - 0.9*c >= 0.8)*u ; c = scan max(0.9*c, e)
  (host analysis: K=10 adds ~3e-3 end-to-end err; 18 is exact). One chain's
  elementwise runs on gpsimd, the other chain + both scans on DVE.
- AllReduces carry bf16 payloads, split in two halves overlapped with the
  producing matmuls.
"""
import os
import numpy as np

import concourse.bass as bass
import concourse.tile as tile
from concourse import bacc, mybir
from concourse.bass_utils import run_bass_kernel_spmd

F32 = mybir.dt.float32
F32R = mybir.dt.float32r
BF16 = mybir.dt.bfloat16
AB = mybir.AluOpType
AFT = mybir.ActivationFunctionType

B, S, DM, H, HKV, L, MLP_MULT = 2, 1024, 1024, 16, 4, 4, 4
DH = DM // H
HASH, VOCAB = 16384, 50257
EPS = 1.1920929e-07
THRESH, DECAY = 0.8, 0.9
ROPE_BASE = 10000.0
N_CORES = 8
TP = 4
HEADS_PC = H // TP        # 4 q heads per core
QD = HEADS_PC * DH        # 256 q dims per core
KD = DH                   # 64 kv dims per core (1 kv head)
HID_PC = MLP_MULT * DM // TP
HASH_PC = HASH // TP
NT = S // 128
ND = DM // 128
KFIX = [9, 9, 9, 9]       # LIF fixpoint scans per layer

_CACHE = {}


def _mm512(nc, psum, lhsT, rhs, start, stop, cols0=0):
    N = rhs.shape[-1]
    for o in range(0, N, 512):
        n = min(512, N - o)
        nc.tensor.matmul(psum[:, cols0 + o:cols0 + o + n], lhsT, rhs[:, o:o + n],
                         start=start, stop=stop)


def build_program():
    nc = bacc.Bacc("TRN2", target_bir_lowering=False, debug=False,
                   enable_asserts=False, num_devices=N_CORES)

    din = {}
    def di(name, shape, dt=F32R):
        din[name] = nc.dram_tensor(name, shape, dt, kind="ExternalInput").ap()
        return din[name]

    xe1 = di("xe1", [DM, S], F32R)
    xe2 = di("xe2", [DM, S], F32R)
    wqkv = di("wqkv", [L, DM, QD + 2 * KD], F32R)   # [WqT|WkT|WvT]
    wo = di("wo", [L, QD, DM])
    wfc = di("wfc", [L, DM, HID_PC])
    wp = di("wp", [L, HID_PC, DM])
    unit = di("unit", [DM, HASH_PC])
    cosq = di("cosq", [128, S], F32)     # q-tile rope tables (2 heads/tile)
    sinq = di("sinq", [128, S], F32)     # signed
    cosk = di("cosk", [64, S], F32)
    sink = di("sink", [64, S], F32)
    pswp = di("pswp", [128, 2, 128], F32R)  # [:,0,:]=Pq ; [0:64,1,0:64]=Pk
    ident = di("ident", [128, 128], F32R)
    tri = di("tri", [128, 128], F32R)
    scal = di("scal", [128, ND, 4 * L], F32)
    qgain = di("qgain", [128, 2, L], F32)
    out_lg = nc.dram_tensor("out_lg", [S, HASH_PC], F32, kind="ExternalOutput").ap()

    # ---------------- persistent SBUF ------------------------------------
    x_t = [nc.alloc_sbuf_tensor(f"x_{d}", [128, S], F32R) for d in range(ND)]
    xn_t = [nc.alloc_sbuf_tensor(f"xn_{d}", [128, S], F32R) for d in range(ND)]
    h_t = [nc.alloc_sbuf_tensor(f"h_{d}", [128, S], F32R) for d in range(ND)]
    qsb = [nc.alloc_sbuf_tensor(f"qsb_{j}", [128, S], F32R) for j in range(2)]
    kvsb = nc.alloc_sbuf_tensor("kvsb", [128, S], F32R)
    q4 = [nc.alloc_sbuf_tensor(f"q4_{j}", [128, S], F32R) for j in range(2)]
    u2 = [nc.alloc_sbuf_tensor(f"u2_{j}", [128, S], F32) for j in range(2)]
    c2 = [nc.alloc_sbuf_tensor(f"c2_{j}", [128, S + 1], F32) for j in range(2)]
    e2 = [nc.alloc_sbuf_tensor(f"e2_{j}", [128, S], F32R) for j in range(2)]
    yt2 = [nc.alloc_sbuf_tensor(f"yt2_{j}", [128, S], F32R) for j in range(2)]
    v65 = nc.alloc_sbuf_tensor("v65", [128, NT, 65], F32R)
    kt2 = nc.alloc_sbuf_tensor("kt2", [128, S], F32R)
    bc_sb = nc.alloc_sbuf_tensor("bc_sb", [128, S], F32)
    cosq_s = nc.alloc_sbuf_tensor("cosq_s", [128, S], F32)
    sinq_s = nc.alloc_sbuf_tensor("sinq_s", [128, S], F32)
    cosk_s = nc.alloc_sbuf_tensor("cosk_s", [64, S], F32)
    sink_s = nc.alloc_sbuf_tensor("sink_s", [64, S], F32)
    pswp_s = nc.alloc_sbuf_tensor("pswp_s", [128, 2, 128], F32R)
    ident_s = nc.alloc_sbuf_tensor("ident_s", [128, 128], F32R)
    tri_s = nc.alloc_sbuf_tensor("tri_s", [128, 128], F32R)
    scal_s = nc.alloc_sbuf_tensor("scal_s", [128, ND, 4 * L], F32)
    qgain_s = nc.alloc_sbuf_tensor("qgain_s", [128, 2, L], F32)
    rkc = nc.alloc_sbuf_tensor("rkc", [128, NT], F32)    # 0.125/rms(k) per key
    lnbc = nc.alloc_sbuf_tensor("lnbc", [128, NT], F32)  # ln(bc) per key
    ibc = nc.alloc_sbuf_tensor("ibc", [128, NT], F32R)    # 1/bc per key
    rows_sb = nc.alloc_sbuf_tensor("rows_sb", [128, S], F32R)
    onesr = nc.alloc_sbuf_tensor("onesr", [128, 128], F32R)
    onesr_f = nc.alloc_sbuf_tensor("onesr_f", [128, 128], F32)
    onesc_f = nc.alloc_sbuf_tensor("onesc_f", [128, 1], F32)
    onesc = nc.alloc_sbuf_tensor("onesc", [128, 1], F32R)
    d9_s = nc.alloc_sbuf_tensor("d9_s", [128, 1], F32)
    mtmp = nc.alloc_sbuf_tensor("mtmp", [128, 1], F32)
    zc = nc.alloc_sbuf_tensor("zc", [128, 1], F32)
    epsc = nc.alloc_sbuf_tensor("epsc", [128, 1], F32)
    rl_row = rows_sb[0:1, :]
    row2f = [bc_sb[0:1, :], bc_sb[32:33, :]]
    ln_row = rows_sb[64:65, :]

    with tile.TileContext(nc) as tc:
        with tc.tile_pool(name="gp", bufs=2) as gp, \
             tc.tile_pool(name="gpb", bufs=2) as gpb, \
             tc.tile_pool(name="gpo", bufs=1) as gpo, \
             tc.tile_pool(name="wq_pool", bufs=2) as wqp, \
             tc.tile_pool(name="wt_pool", bufs=3) as wtp, \
             tc.tile_pool(name="psB", bufs=4, space="PSUM") as psB, \
             tc.tile_pool(name="dram", bufs=1, space="DRAM") as dram:

            arb_i = dram.tile([DM, S], F32R)
            arb_p = dram.tile([DM, S], BF16)
            arb_o = dram.tile([DM, S], BF16)

            # ---- constants / tables ----
            nc.sync.dma_start(cosq_s[:], cosq[:])
            nc.sync.dma_start(sinq_s[:], sinq[:])
            nc.sync.dma_start(cosk_s[:], cosk[:])
            nc.sync.dma_start(sink_s[:], sink[:])
            nc.sync.dma_start(pswp_s[:], pswp[:])
            nc.sync.dma_start(ident_s[:], ident[:])
            nc.sync.dma_start(tri_s[:], tri[:])
            nc.sync.dma_start(scal_s[:], scal[:])
            nc.sync.dma_start(qgain_s[:], qgain[:])
            nc.vector.memset(mtmp[:], 1.0)
            nc.vector.tensor_copy(onesc[:], mtmp[:])
            nc.vector.tensor_copy(onesr[:], mtmp[:, 0:1].to_broadcast((128, 128)))
            nc.vector.tensor_copy(onesr_f[:], mtmp[:, 0:1].to_broadcast((128, 128)))
            nc.vector.tensor_copy(onesc_f[:], mtmp[:])
            nc.vector.memset(d9_s[:], 0.9)
            nc.vector.memset(zc[:], 0.0)
            nc.vector.memset(epsc[:], EPS)

            # ---- embedding: x = xe1 + xe2 (also x0, kept in DRAM) ----
            for d in range(ND):
                t1 = gp.tile([128, S], F32R, tag="gp")
                nc.sync.dma_start(t1[:], xe1[128 * d:128 * d + 128, :])
                nc.sync.dma_start(x_t[d][:], xe2[128 * d:128 * d + 128, :])
                nc.vector.scalar_tensor_tensor(x_t[d][:], x_t[d][:], 1.0, t1[:],
                                               AB.mult, AB.add)
                nc.sync.dma_start(arb_i[128 * d:128 * d + 128, :], x_t[d][:])

            def ssq_row(ps):
                """ps[0:1,:] = sum over DM of x^2 (per token)."""
                for d in range(ND):
                    sq = gp.tile([128, S], F32R, tag="gp")
                    nc.scalar.activation(sq[:], x_t[d][:], AFT.Square,
                                         bias=zc[:])
                    _mm512(nc, ps[0:1, :], onesc[:], sq[:],
                           start=(d == 0), stop=(d == ND - 1))

            for l in range(L):
                # ---- resid mix: x = rm0*x + rm1*x0 ----
                for d in range(ND):
                    x0t = gp.tile([128, S], F32R, tag="gp")
                    nc.sync.dma_start(x0t[:], arb_i[128 * d:128 * d + 128, :])
                    rm0 = scal_s[:, d, 4 * l + 0:4 * l + 1]
                    rm1 = scal_s[:, d, 4 * l + 1:4 * l + 2]
                    tt = gp.tile([128, S], F32, tag="gp")
                    nc.scalar.mul(tt[:], x0t[:], rm1)
                    nc.vector.scalar_tensor_tensor(x_t[d][:], x_t[d][:], rm0,
                                                   tt[:], AB.mult, AB.add)

                # ---- per-token ln(rsqrt(mean x^2 + eps)) for v (exp bias) ----
                ssq_ps = psB.tile([128, S], F32, tag="psB")
                ssq_row(ssq_ps)
                nc.scalar.activation(ln_row, ssq_ps[0:1, :], AFT.Ln,
                                     bias=epsc[0:1, :], scale=1.0 / DM)
                nc.vector.tensor_scalar(ln_row, ln_row, -0.5, None, AB.mult)
                lnp = psB.tile([128, S], F32, tag="psB")
                for t in range(NT):
                    nc.tensor.transpose(lnp[:, t:t + 1],
                                        rows_sb[64:65, 128 * t:128 * t + 128]
                                        .bitcast(F32),
                                        ident_s[64:65, 64:65].bitcast(F32))
                nc.scalar.copy(lnbc[:], lnp[:, 0:NT])
                # 1/bc per key (denominator weights: et carries a bc factor)
                nc.scalar.activation(ibc[:], lnbc[:], AFT.Exp,
                                     bias=zc[:], scale=-1.0)

                # ---- QKV projection (f32r, transposed out: [dim, tokens]) --
                pss = [psB.tile([128, S], F32, tag="psB", name=f"qkvp{i}")
                       for i in range(2)]
                pss.append(psB.tile([128, S], F32, tag="psB", name="qkvp2"))
                for ch in range(2):
                    for d in range(ND):
                        wt = wqp.tile([128, QD + 2 * KD], F32R, tag="wq")
                        nc.sync.dma_start(wt[:], wqkv[l, 128 * d:128 * d + 128, :])
                        xr = x_t[d][:, 512 * ch:512 * ch + 512]
                        for jt in range(3):
                            nc.tensor.matmul(
                                pss[jt][:, 512 * ch:512 * ch + 512],
                                wt[:, 128 * jt:128 * jt + 128], xr,
                                start=(d == 0), stop=(d == ND - 1))
                    for jt in range(2):
                        nc.scalar.copy(qsb[jt][:, 512 * ch:512 * ch + 512],
                                       pss[jt][:, 512 * ch:512 * ch + 512])
                    nc.scalar.copy(kvsb[:, 512 * ch:512 * ch + 512],
                                   pss[2][:, 512 * ch:512 * ch + 512])

                # ---- q-head rms + rope (rms divide from psum broadcast) ----
                for jt in range(2):
                    sq = gp.tile([128, S], F32R, tag="gp")
                    nc.scalar.activation(sq[:], qsb[jt][:], AFT.Square,
                                         bias=zc[:])
                    rqb = psB.tile([128, S], F32, tag="psB")
                    for hh in range(2):
                        rq_ps = psB.tile([128, S], F32, tag="psB")
                        _mm512(nc, rq_ps[0:1, :],
                               onesc[64 * hh:64 * hh + 64, :],
                               sq[64 * hh:64 * hh + 64, :],
                               start=True, stop=True)
                        nc.scalar.activation(row2f[hh], rq_ps[0:1, :],
                                             AFT.Sqrt, bias=zc[0:1, :],
                                             scale=1.0 / DH)
                        _mm512(nc, rqb[64 * hh:64 * hh + 64, :],
                               onesr_f[32 * hh:32 * hh + 1, 0:64],
                               row2f[hh],
                               start=True, stop=True)
                    rqi = gp.tile([128, S], F32, tag="gp", name="rqi")
                    nc.vector.reciprocal_approx_fast(out=rqi[:], in_=rqb[:])
                    for ch in range(2):
                        cs = slice(512 * ch, 512 * ch + 512)
                        swp = psB.tile([128, S], F32, tag="psB")
                        nc.tensor.matmul(swp[:, 0:512], pswp_s[:, 0, :],
                                         qsb[jt][:, cs],
                                         start=True, stop=True)
                        t1 = e2[0][:, cs]
                        nc.vector.scalar_tensor_tensor(
                            t1, qsb[jt][:, cs], 1.0, cosq_s[:, cs],
                            AB.mult, AB.mult)
                        t2 = e2[1][:, cs]
                        nc.vector.scalar_tensor_tensor(
                            t2, swp[:, 0:512], 1.0, sinq_s[:, cs],
                            AB.mult, AB.mult)
                        nc.vector.scalar_tensor_tensor(
                            t1, t1, 1.0, t2, AB.mult, AB.add)
                        nc.vector.scalar_tensor_tensor(
                            qsb[jt][:, cs], t1, 1.0,
                            rqi[:, cs], AB.mult, AB.mult)
                for ch in range(2):
                    cs = slice(512 * ch, 512 * ch + 512)
                    swp = psB.tile([128, S], F32, tag="psB")
                    nc.tensor.matmul(swp[0:64, 0:512], pswp_s[0:64, 1, 0:64],
                                     kvsb[0:64, cs],
                                     start=True, stop=True)
                    t1 = e2[0][0:64, cs]
                    nc.vector.scalar_tensor_tensor(
                        t1, kvsb[0:64, cs], 1.0, cosk_s[:, cs], AB.mult, AB.mult)
                    t2 = e2[1][0:64, cs]
                    nc.vector.scalar_tensor_tensor(
                        t2, swp[0:64, 0:512], 1.0, sink_s[:, cs],
                        AB.mult, AB.mult)
                    nc.vector.scalar_tensor_tensor(
                        kt2[0:64, cs], t1, 1.0, t2, AB.mult, AB.add)
                    nc.scalar.copy(kt2[64:128, cs], kt2[0:64, cs])

                # ---- k-head rms -> per-key scale column (0.125/rms) ----
                ksq = gp.tile([128, S], F32, tag="gp")
                nc.scalar.activation(ksq[0:64, :], kt2[0:64, :], AFT.Square,
                                     bias=zc[0:64, :])
                rkp = psB.tile([128, S], F32, tag="psB")
                for t in range(NT):
                    nc.tensor.matmul(rkp[:, t:t + 1],
                                     ksq[0:64, 128 * t:128 * t + 128],
                                     onesc_f[0:64, :],
                                     start=True, stop=True)
                nc.scalar.activation(rkc[:], rkp[:, 0:NT], AFT.Sqrt,
                                     bias=zc[:], scale=1.0 / DH)
                nc.vector.reciprocal_approx_fast(out=rkc[:], in_=rkc[:])
                nc.vector.tensor_scalar(rkc[:], rkc[:], 0.125, None, AB.mult)

                # ---- v -> token-major tiles via PE transpose ----
                for t in range(NT):
                    vtp = psB.tile([128, S], F32, tag="psB")
                    nc.tensor.transpose(vtp[:, 0:64].bitcast(F32R),
                                        kvsb[64:128, 128 * t:128 * t + 128],
                                        ident_s[64:128, 0:64])
                    nc.scalar.copy(v65[:, t, 0:64], vtp[:, 0:64])
                    nc.vector.tensor_copy(v65[:, t, 64:65], ibc[:, t:t + 1])

                # ---- LIF: u scan + fixpoint (K scans) ----
                d9 = d9_s[:].to_broadcast((128, S))
                for j in range(2):
                    nc.vector.tensor_tensor_scan(u2[j][:], d9, qsb[j][:], 0.0,
                                                 AB.mult, AB.add)
                    nc.vector.memset(c2[j][:, 0:1], 0.0)
                for p in range(KFIX[l]):
                    for j in range(2):
                        eng = nc.vector
                        if p == 0:
                            eng.scalar_tensor_tensor(
                                e2[j][:], u2[j][:], THRESH, u2[j][:],
                                AB.is_ge, AB.mult)
                        else:
                            eng.scalar_tensor_tensor(
                                e2[j][:], c2[j][:, 0:S], -DECAY, u2[j][:],
                                AB.mult, AB.add)
                            eng.scalar_tensor_tensor(
                                e2[j][:], e2[j][:], THRESH, u2[j][:],
                                AB.is_ge, AB.mult)
                        nc.vector.tensor_tensor_scan(
                            c2[j][:, 1:S + 1], d9, e2[j][:], 0.0, AB.mult, AB.max)
                # final spikes*gain -> c2[:,0:S]; gated q -> q4
                for j in range(2):
                    nc.vector.scalar_tensor_tensor(
                        e2[j][:], c2[j][:, 0:S], -DECAY, u2[j][:], AB.mult, AB.add)
                    nc.vector.tensor_scalar(c2[j][:, 0:S], e2[j][:], THRESH,
                                            qgain_s[:, j, l:l + 1],
                                            AB.is_ge, AB.mult)
                    eng = nc.vector
                    eng.scalar_tensor_tensor(q4[j][:], qsb[j][:], 1.0,
                                             c2[j][:, 0:S], AB.mult, AB.mult)

                # ---- attention (heads sequential; per-head yup/dn) ----
                for j in range(2):
                    yups = [psB.tile([128, S], F32, tag="psB", name=f"yup{i}")
                            for i in range(2)]
                    ets = [e2[0][:], e2[1][:], qsb[0][:], qsb[1][:]]
                    for hl in range(2):
                        hh, off = 2 * j + hl, 64 * hl
                        yup = yups[hl]
                        for t in range(NT):
                            ncols = S - 128 * t
                            et = ets[t % 4]
                            scp = psB.tile([128, S], F32, tag="psB")
                            _mm512(nc, scp,
                                   kt2[off:off + 64, 128 * t:128 * t + 128],
                                   q4[j][off:off + 64, 128 * t:S],
                                   start=True, stop=True)
                            nc.scalar.activation(
                                et[:, 0:ncols], scp[:, 0:ncols],
                                AFT.Exp, bias=lnbc[:, t:t + 1],
                                scale=rkc[:, t:t + 1])
                            nc.vector.scalar_tensor_tensor(
                                et[:, 0:128], et[:, 0:128], 1.0,
                                tri_s[:], AB.mult, AB.mult)
                            _mm512(nc, yup[0:65, :], v65[:, t, :],
                                   et[:, 0:ncols],
                                   start=(t == 0), stop=(t == NT - 1),
                                   cols0=128 * t)
                        nc.scalar.copy(u2[j][off:off + 64, :], yup[0:64, :])
                        nc.scalar.copy(row2f[hl], yup[64:65, :])
                    # epilogue for chain j: divide by denominator broadcast
                    rbp = psB.tile([128, S], F32, tag="psB")
                    for hl in range(2):
                        _mm512(nc, rbp[64 * hl:64 * hl + 64, :],
                               onesr_f[32 * hl:32 * hl + 1, 0:64],
                               row2f[hl],
                               start=True, stop=True)
                    rbi = gp.tile([128, S], F32, tag="gp", name="rbi")
                    nc.vector.reciprocal_approx_fast(out=rbi[:], in_=rbp[:])
                    nc.vector.scalar_tensor_tensor(yt2[j][:], u2[j][:], 1.0,
                                                   rbi[:], AB.mult, AB.mult)

                # ---- Wo -> bf16 partials -> chunked AllReduce ----
                for d in range(ND):
                    aop = psB.tile([128, S], F32, tag="psB")
                    wt = wtp.tile([128, 2, 128], F32R, tag="wblk")
                    nc.sync.dma_start(
                        wt[:], wo[l, :, 128 * d:128 * d + 128].rearrange(
                            "(c p) f -> p c f", p=128))
                    for c in range(2):
                        _mm512(nc, aop, wt[:, c, :], yt2[c][:], start=(c == 0),
                               stop=(c == 1))
                    att = gpb.tile([128, S], BF16, tag="gpb")
                    nc.scalar.copy(att[:], aop[:])
                    nc.sync.dma_start(arb_p[128 * d:128 * d + 128, :], att[:])
                    if d == 3:
                        nc.gpsimd.collective_compute(
                            "AllReduce", AB.add,
                            replica_groups=[[0, 1, 2, 3], [4, 5, 6, 7]],
                            ins=[arb_p[0:512, :].opt()],
                            outs=[arb_o[0:512, :].opt()])
                nc.gpsimd.collective_compute(
                    "AllReduce", AB.add,
                    replica_groups=[[0, 1, 2, 3], [4, 5, 6, 7]],
                    ins=[arb_p[512:1024, :].opt()],
                    outs=[arb_o[512:1024, :].opt()])
                for d in range(ND):
                    att = gpb.tile([128, S], BF16, tag="gpb")
                    nc.sync.dma_start(att[:], arb_o[128 * d:128 * d + 128, :])
                    asc = scal_s[:, d, 4 * l + 2:4 * l + 3]
                    eng = nc.vector
                    eng.scalar_tensor_tensor(x_t[d][:], att[:], asc, x_t[d][:],
                                             AB.mult, AB.add)

                # ---- MLP rmsnorm (materialized xn) ----
                ssq_ps = psB.tile([128, S], F32, tag="psB")
                ssq_row(ssq_ps)
                nc.scalar.activation(rl_row, ssq_ps[0:1, :], AFT.Sqrt,
                                     bias=epsc[0:1, :], scale=1.0 / DM)
                nc.vector.reciprocal_approx_fast(
                    out=rl_row.bitcast(F32), in_=rl_row.bitcast(F32))
                bcp = psB.tile([128, S], F32, tag="psB")
                _mm512(nc, bcp, onesr_f[0:1, :], rl_row.bitcast(F32),
                       start=True, stop=True)
                nc.scalar.copy(bc_sb[:], bcp[:])
                for d in range(ND):
                    eng = nc.vector
                    eng.scalar_tensor_tensor(xn_t[d][:], x_t[d][:], 1.0,
                                             bc_sb[:], AB.mult, AB.mult)

                # ---- MLP ----
                for hh in range(ND):
                    hp = psB.tile([128, S], F32, tag="psB")
                    for g in range(2):
                        wt = wtp.tile([128, 4, 128], F32R, tag="wblk")
                        nc.sync.dma_start(
                            wt[:], wfc[l, 512 * g:512 * g + 512,
                                       128 * hh:128 * hh + 128].rearrange(
                                "(dd p) f -> p dd f", p=128))
                        for dd in range(4):
                            d = 4 * g + dd
                            _mm512(nc, hp, wt[:, dd, :], xn_t[d][:],
                                   start=(d == 0), stop=(d == ND - 1))
                    hraw = gp.tile([128, S], F32, tag="gp")
                    nc.scalar.copy(hraw[:], hp[:])
                    eng = nc.vector
                    hm = gp.tile([128, S], F32, tag="gp")
                    eng.tensor_scalar(hm[:], hraw[:], 0.0, 0.01, AB.min, AB.mult)
                    eng.scalar_tensor_tensor(h_t[hh][:], hraw[:], 0.0, hraw[:],
                                             AB.max, AB.mult)
                    eng.scalar_tensor_tensor(h_t[hh][:], h_t[hh][:], 1.0,
                                             hm[:], AB.mult, AB.add)
                for d in range(ND):
                    mlpp = psB.tile([128, S], F32, tag="psB")
                    for g in range(2):
                        wt = wtp.tile([128, 4, 128], F32R, tag="wblk")
                        nc.sync.dma_start(
                            wt[:], wp[l, 512 * g:512 * g + 512,
                                      128 * d:128 * d + 128].rearrange(
                                "(dd p) f -> p dd f", p=128))
                        for dd in range(4):
                            hh = 4 * g + dd
                            _mm512(nc, mlpp, wt[:, dd, :], h_t[hh][:],
                                   start=(hh == 0), stop=(hh == ND - 1))
                    mt = gpb.tile([128, S], BF16, tag="gpb")
                    nc.scalar.copy(mt[:], mlpp[:])
                    nc.sync.dma_start(arb_p[128 * d:128 * d + 128, :], mt[:])
                    if d == 3:
                        nc.gpsimd.collective_compute(
                            "AllReduce", AB.add,
                            replica_groups=[[0, 1, 2, 3], [4, 5, 6, 7]],
                            ins=[arb_p[0:512, :].opt()],
                            outs=[arb_o[0:512, :].opt()])
                nc.gpsimd.collective_compute(
                    "AllReduce", AB.add,
                    replica_groups=[[0, 1, 2, 3], [4, 5, 6, 7]],
                    ins=[arb_p[512:1024, :].opt()],
                    outs=[arb_o[512:1024, :].opt()])
                for d in range(ND):
                    mt = gpb.tile([128, S], BF16, tag="gpb")
                    nc.sync.dma_start(mt[:], arb_o[128 * d:128 * d + 128, :])
                    msc = scal_s[:, d, 4 * l + 3:4 * l + 4]
                    eng = nc.vector
                    eng.scalar_tensor_tensor(x_t[d][:], mt[:], msc, x_t[d][:],
                                             AB.mult, AB.add)

            # ---- final norm + logits ----
            ssq_ps = psB.tile([128, S], F32, tag="psB")
            ssq_row(ssq_ps)
            nc.scalar.activation(rl_row, ssq_ps[0:1, :], AFT.Sqrt,
                                 bias=epsc[0:1, :], scale=1.0 / DM)
            nc.vector.reciprocal_approx_fast(
                out=rl_row.bitcast(F32), in_=rl_row.bitcast(F32))
            bcp = psB.tile([128, S], F32, tag="psB")
            _mm512(nc, bcp, onesr_f[0:1, :], rl_row.bitcast(F32),
                   start=True, stop=True)
            nc.scalar.copy(bc_sb[:], bcp[:])
            for d in range(ND):
                eng = nc.vector
                eng.scalar_tensor_tensor(xn_t[d][:], x_t[d][:], 1.0,
                                         bc_sb[:], AB.mult, AB.mult)
            for o in range(HASH_PC // 512):
                for tg in range(2):
                    lg_ps = [psB.tile([128, S], F32, tag="psB", name=f"lgp{i}")
                             for i in range(2)]
                    for d in range(ND):
                        ut = wtp.tile([128, 512], F32R, tag="wblk")
                        nc.sync.dma_start(ut[:],
                                          unit[128 * d:128 * d + 128,
                                               512 * o:512 * o + 512])
                        for ti in range(4):
                            t = 4 * tg + ti
                            nc.tensor.matmul(
                                lg_ps[ti // 2][:, 512 * (ti % 2):512 * (ti % 2) + 512],
                                xn_t[d][:, 128 * t:128 * t + 128],
                                ut[:], start=(d == 0), stop=(d == ND - 1))
                    for ti in range(4):
                        t = 4 * tg + ti
                        ot = gpo.tile([128, 512], F32, tag="gpo")
                        nc.scalar.copy(
                            ot[:],
                            lg_ps[ti // 2][:, 512 * (ti % 2):512 * (ti % 2) + 512])
                        nc.sync.dma_start(out_lg[128 * t:128 * t + 128,
                                                 512 * o:512 * o + 512], ot[:])

    nc.compile()
    return nc


def _host_prep(inputs):
    ids = np.asarray(inputs["input_ids"])
    uni = np.ascontiguousarray(inputs["uni"], np.float32)
    bi = np.ascontiguousarray(inputs["bi"], np.float32)
    Wq = np.asarray(inputs["Wq"], dtype=np.float32)
    Wk = np.asarray(inputs["Wk"], dtype=np.float32)
    Wv = np.asarray(inputs["Wv"], dtype=np.float32)
    Wo = np.asarray(inputs["Wo"], dtype=np.float32)
    Wfc = np.asarray(inputs["Wfc"], dtype=np.float32)
    Wp = np.asarray(inputs["Wp"], dtype=np.float32)
    qg = np.asarray(inputs["q_gain"], dtype=np.float32)
    asc = np.asarray(inputs["attn_scale"], dtype=np.float32)
    msc = np.asarray(inputs["mlp_scale"], dtype=np.float32)
    rmx = np.asarray(inputs["resid_mix"], dtype=np.float32)

    prev = np.concatenate([np.zeros_like(ids[:, :1]), ids[:, :-1]], axis=1)
    h1 = (ids % HASH).astype(np.int64)
    h2 = ((prev.astype(np.int64) * 31 + ids) % HASH).astype(np.int64)

    inv_freq = 1.0 / (ROPE_BASE ** (np.arange(0, DH, 2, dtype=np.float32) / DH))
    freqs = np.arange(S, dtype=np.float32)[:, None] * inv_freq[None, :]
    cos = np.cos(freqs).astype(np.float32)   # [S, 32]
    sin = np.sin(freqs).astype(np.float32)
    cos64 = np.ascontiguousarray(np.concatenate([cos, cos], axis=1).T)  # [64,S]
    sin64 = np.ascontiguousarray(np.concatenate([sin, -sin], axis=1).T)
    cosq = np.ascontiguousarray(np.tile(cos64, (2, 1)))   # [128, S]
    sinq = np.ascontiguousarray(np.tile(sin64, (2, 1)))

    # swap permutations: P~[k, m] = 1 iff k = partner(m) (partner: +-32 in 64)
    pswp = np.zeros((128, 2, 128), np.float32)
    for m in range(128):
        base = (m // 64) * 64
        partner = base + (m % 64 + 32) % 64
        pswp[partner, 0, m] = 1.0
    for m in range(64):
        pswp[(m + 32) % 64, 1, m] = 1.0
    ident = np.eye(128, dtype=np.float32)
    ident[64:128, 0:64] += np.eye(64, dtype=np.float32)
    trim = np.tril(np.ones((128, 128), np.float32)).T.copy()

    scal = np.zeros((128, ND, 4 * L), np.float32)
    for l in range(L):
        for v, vec in enumerate((rmx[l, 0], rmx[l, 1], asc[l], msc[l])):
            scal[:, :, 4 * l + v] = vec.reshape(ND, 128).T

    in_maps = []
    for core in range(N_CORES):
        g, r = core // TP, core % TP
        qsl = slice(QD * r, QD * (r + 1))
        ksl = slice(KD * r, KD * (r + 1))
        hsl = slice(HID_PC * r, HID_PC * (r + 1))
        asl = slice(HASH_PC * r, HASH_PC * (r + 1))
        wqkv = np.concatenate([
            Wq[:, qsl, :].transpose(0, 2, 1),
            Wk[:, ksl, :].transpose(0, 2, 1),
            Wv[:, ksl, :].transpose(0, 2, 1)], axis=2)
        qgain = np.zeros((128, 2, L), np.float32)
        for l in range(L):
            for j in range(2):
                for hp in range(2):
                    head = HEADS_PC * r + 2 * j + hp
                    qgain[64 * hp:64 * hp + 64, j, l] = qg[l, head]
        m = dict(
            xe1=np.ascontiguousarray(uni[h1[g]].T),
            xe2=np.ascontiguousarray(bi[h2[g]].T),
            wqkv=np.ascontiguousarray(wqkv),
            wo=np.ascontiguousarray(Wo[:, :, qsl].transpose(0, 2, 1)),
            wfc=np.ascontiguousarray(Wfc[:, hsl, :].transpose(0, 2, 1)),
            wp=np.ascontiguousarray(Wp[:, :, hsl].transpose(0, 2, 1)),
            unit=np.ascontiguousarray(uni[asl, :].T),
            cosq=cosq,
            sinq=sinq,
            cosk=cos64,
            sink=sin64,
            pswp=pswp,
            ident=ident,
            tri=trim,
            scal=scal,
            qgain=qgain,
        )
        in_maps.append(m)
    return in_maps


def kernel(**inputs):
    if "nc" not in _CACHE:
        _CACHE["nc"] = build_program()
    nc = _CACHE["nc"]
    in_maps = _host_prep(inputs)
    res = run_bass_kernel_spmd(nc, in_maps, core_ids=list(range(N_CORES)),
                               trace=os.environ.get("K_TRACE", "0") == "1")
    _CACHE["res"] = res
    out = np.zeros((B, S, HASH), np.float32)
    for core in range(N_CORES):
        g, r = core // TP, core % TP
        out[g, :, HASH_PC * r:HASH_PC * (r + 1)] = res.results[core]["out_lg"]
    return out

